# revision 26
# baseline (speedup 1.0000x reference)
"""Trainium2 Bass kernel for nn_DepPairingLayer (bidirectional chain-TreeLSTM over
shortest-path node chains + span mean-pooling + pair MLP), SPMD across 8 NeuronCores.

Sharding: data-parallel over the pair dimension P=8192 (1024 pairs/core, which is
exactly 4 batches x 256 pairs for the span pooling); all weights replicated.

Device layout is feature-major: activations live as [features(partitions), pairs(free)]
so the LSTM recurrence h @ U and the input projection x @ W become matmuls with the
weights as the stationary operand ([in_dim, out_dim] blocks) and the data as the
moving operand. All matmuls run in bf16 with fp32 PSUM accumulation (measured
end-to-end rel-absmax error vs the fp32 reference: ~4e-3).

Host-side prep (layout/cast only): node_embs is pre-transposed to [L, D, pairs] and
cast to bf16 so per-step slices DMA as dense [128, C] tiles; weights are concatenated
([Wiou|Wf] -> [D, 4H]) and cast; span start/end/recip scalars are laid out
partition-major.
"""

from contextlib import ExitStack

import numpy as np
import ml_dtypes

import concourse.bass as bass
import concourse.mybir as mybir
import concourse.tile as tile
from concourse import bacc
from concourse.bass_utils import run_bass_kernel_spmd
from concourse.masks import make_identity

bf16 = ml_dtypes.bfloat16
FP32 = mybir.dt.float32
BF16 = mybir.dt.bfloat16
ALU = mybir.AluOpType
ACTF = mybir.ActivationFunctionType

# problem dims (hardcoded per contract)
NCORES = 8
B, PB, L, D, H, DT, T = 32, 256, 16, 832, 384, 768, 512
P = B * PB                      # 8192 pairs
PS = P // NCORES                # 1024 pairs per core
NB = B // NCORES                # 4 batches per core
C = 512                         # pair-chunk (matmul moving free dim)
NCH = PS // C                   # 2 chunks per core
H4 = 4 * H                      # 1536 = i|o|u|f
# D=832 is NOT a multiple of 128: 6 full k-tiles + one 64-row tile
KD_TILES = [(i * 128, 128) for i in range(D // 128)] + (
    [(D - D % 128, D % 128)] if D % 128 else [])
KD = len(KD_TILES)              # 7 k-tiles of node features
M12 = H4 // 128                 # 12 m-tiles of gate features
KH = H // 128                   # 3 k-tiles of hidden
DEC_IN, DEC_H, DEC_OUT = 3 * H + 2 * DT, 512, 7
K21 = DEC_IN // 128             # 21 feature k-tiles for W1
M4 = DEC_H // 128               # 4 m-tiles for W1 output
MT = DT // 128                  # 6 span-feature m-tiles
JT = PB // 128                  # 2 pair-tiles per batch (for masks)


def _build_program(debug: bool = False) -> bass.Bass:
    nc = bacc.Bacc("TRN2", target_bir_lowering=False, debug=False,
                   num_devices=NCORES)
    dp = nc.declare_dram_parameter
    if debug:
        dbg_span = dp("dbg_span", [2, MT, 128, PS], BF16, isOutput=True)
        dbg_racc = dp("dbg_racc", [NCH, KH, 128, C], BF16, isOutput=True)
        dbg_start = dp("dbg_start", [NCH, KH, 128, C], BF16, isOutput=True)
        dbg_end = dp("dbg_end", [NCH, KH, 128, C], BF16, isOutput=True)
        dbg_g0 = dp("dbg_g0", [M12, 128, C], BF16, isOutput=True)
        dbg_h1 = dp("dbg_h1", [KH, 128, C], BF16, isOutput=True)
        dbg_nd = dp("dbg_nd", [D, C], BF16, isOutput=True)
        dbg_pm = dp("dbg_pm", [128, C], FP32, isOutput=True)

    node_T = dp("node_T", [L, D, PS], BF16, isOutput=False)
    tok = dp("tok", [NB, T, DT], BF16, isOutput=False)
    root = dp("root", [1, PS], FP32, isOutput=False)
    sp_all = dp("sp_all", [2, NB, JT, 128, 4], FP32, isOutput=False)
    Wu = dp("Wu", [D, H4], BF16, isOutput=False)
    Wd = dp("Wd", [D, H4], BF16, isOutput=False)
    Uu = dp("Uu", [H, H4], BF16, isOutput=False)
    Ud = dp("Ud", [H, H4], BF16, isOutput=False)
    W1 = dp("W1", [DEC_IN, DEC_H], BF16, isOutput=False)
    W2 = dp("W2", [DEC_H, DEC_OUT], BF16, isOutput=False)
    bu = dp("bu", [M12, 128, 1], FP32, isOutput=False)
    bd = dp("bd", [M12, 128, 1], FP32, isOutput=False)
    b1 = dp("b1", [M4, 128, 1], FP32, isOutput=False)
    b2 = dp("b2", [DEC_OUT, 1], FP32, isOutput=False)
    ones = dp("ones", [1, 128], BF16, isOutput=False)
    iota_d = dp("iota_d", [128, T], FP32, isOutput=False)
    out_d = dp("out", [DEC_OUT, PS], FP32, isOutput=True)

    def loadc(pool, name, src_ap, shape, dtype, bufs=1):
        t = pool.tile(shape, dtype, name=name, tag=name, bufs=bufs)
        nc.sync.dma_start(t[:], src_ap)
        return t

    with tile.TileContext(nc) as tc, ExitStack() as ctx:
        # whole-program pools
        cpool = ctx.enter_context(tc.tile_pool(name="const", bufs=1))
        spanp = ctx.enter_context(tc.tile_pool(name="spanp", bufs=1))
        capp = ctx.enter_context(tc.tile_pool(name="capp", bufs=1))
        pmm = ctx.enter_context(tc.tile_pool(name="pmm", bufs=4, space="PSUM"))
        ptp = ctx.enter_context(tc.tile_pool(name="ptp", bufs=2, space="PSUM"))
        pmask = ctx.enter_context(tc.tile_pool(name="pmask", bufs=1, space="PSUM"))
        pout = ctx.enter_context(tc.tile_pool(name="pout", bufs=1, space="PSUM"))

        bu_t = [loadc(cpool, f"bu{m}", bu[m], [128, 1], FP32) for m in range(M12)]
        bd_t = [loadc(cpool, f"bd{m}", bd[m], [128, 1], FP32) for m in range(M12)]
        b1_t = [loadc(cpool, f"b1{m}", b1[m], [128, 1], FP32) for m in range(M4)]
        b2_t = loadc(cpool, "b2t", b2[:, :], [DEC_OUT, 1], FP32)
        ones_t = loadc(cpool, "onest", ones[:, :], [1, 128], BF16)
        root_t = loadc(cpool, "roott", root[:, :], [1, PS], FP32)
        iota_t = loadc(cpool, "iota", iota_d[:, :], [128, T], FP32)
        ident = cpool.tile([128, 128], BF16, name="ident", tag="ident")
        make_identity(nc, ident[:])

        b_t = {"u": bu_t, "d": bd_t}

        # spanT[sp][m]: [128, PS] bf16 feature-major span means (whole program)
        spanT = [[spanp.tile([128, PS], BF16, name=f"span{sp}_{m}",
                             tag=f"span{sp}_{m}") for m in range(MT)]
                 for sp in range(2)]
        # per-chunk LSTM summary tiles (whole program; consumed by the MLP phase)
        root_acc = [[capp.tile([128, C], BF16, name=f"racc{ch}_{k}",
                               tag=f"racc{ch}_{k}") for k in range(KH)]
                    for ch in range(NCH)]
        start_t = [[None] * KH for _ in range(NCH)]
        end_t = [[None] * KH for _ in range(NCH)]

        # ---- phase 1: span mean pooling --------------------------------
        with tc.tile_pool(name="tokp", bufs=2) as tokp, \
             tc.tile_pool(name="mwork", bufs=2) as mwork:
            for b in range(NB):
                tk = []
                for tb in range(T // 128):
                    t = tokp.tile([128, DT], BF16, name=f"tok{tb}", tag=f"tok{tb}")
                    nc.sync.dma_start(t[:], tok[b, tb * 128:(tb + 1) * 128, :])
                    tk.append(t)
                for sp in range(2):
                    maskT = [mwork.tile([128, PB], BF16, name=f"mT{tb}",
                                        tag=f"mT{tb}") for tb in range(T // 128)]
                    for jt in range(JT):
                        sc3 = mwork.tile([128, 4], FP32, name="sc3", tag="sc3",
                                         bufs=4)
                        nc.sync.dma_start(sc3[:], sp_all[sp, b, jt])
                        cmp1 = mwork.tile([128, T], BF16, name="cmp1", tag="cmp1")
                        cmp2 = mwork.tile([128, T], BF16, name="cmp2", tag="cmp2")
                        nc.vector.tensor_scalar(cmp1[:], iota_t[:], sc3[:, 0:1],
                                                None, ALU.is_ge)
                        nc.vector.tensor_scalar(cmp2[:], iota_t[:], sc3[:, 1:2],
                                                None, ALU.is_lt)
                        m16 = mwork.tile([128, T], BF16, name="m16", tag="m16")
                        nc.vector.scalar_tensor_tensor(m16[:], cmp1[:], sc3[:, 2:3],
                                                       cmp2[:], op0=ALU.mult,
                                                       op1=ALU.mult)
                        for tb in range(T // 128):
                            tp = ptp.tile([128, 128], BF16, name="tp", tag="tp")
                            nc.tensor.transpose(
                                tp[:], m16[:, tb * 128:(tb + 1) * 128], ident[:])
                            nc.vector.tensor_copy(
                                maskT[tb][:, jt * 128:(jt + 1) * 128], tp[:])
                    for m in range(MT):
                        zp = pmm.tile([128, PB], FP32, name="zp", tag="mm")
                        for tb in range(T // 128):
                            nc.tensor.matmul(zp[:], tk[tb][:, m * 128:(m + 1) * 128],
                                             maskT[tb][:], start=(tb == 0),
                                             stop=(tb == T // 128 - 1))
                        nc.vector.tensor_copy(spanT[sp][m][:, b * PB:(b + 1) * PB],
                                              zp[:])

        # ---- phase 2: bidirectional chain-LSTM per pair-chunk ----------
        with tc.tile_pool(name="lstmw", bufs=1) as lstmw, \
             tc.tile_pool(name="nodep", bufs=3) as nodep, \
             tc.tile_pool(name="statep", bufs=2) as statep, \
             tc.tile_pool(name="gatep", bufs=16) as gatep, \
             tc.tile_pool(name="eqp", bufs=4) as eqp:
            wu_t = [loadc(lstmw, f"wu{k}", Wu[ko:ko + ksz, :], [ksz, H4], BF16)
                    for k, (ko, ksz) in enumerate(KD_TILES)]
            wd_t = [loadc(lstmw, f"wd{k}", Wd[ko:ko + ksz, :], [ksz, H4], BF16)
                    for k, (ko, ksz) in enumerate(KD_TILES)]
            uu_t = [loadc(lstmw, f"uu{k}", Uu[k * 128:(k + 1) * 128, :], [128, H4],
                          BF16) for k in range(KH)]
            ud_t = [loadc(lstmw, f"ud{k}", Ud[k * 128:(k + 1) * 128, :], [128, H4],
                          BF16) for k in range(KH)]
            w_t = {"u": wu_t, "d": wd_t}
            u_t = {"u": uu_t, "d": ud_t}

            for ch in range(NCH):
                c0 = ch * C
                h16 = {}
                cst = {}
                for d in ("u", "d"):
                    h16[d] = [statep.tile([128, C], BF16, name=f"h_{d}{k}",
                                          tag=f"h_{d}{k}") for k in range(KH)]
                    cst[d] = [statep.tile([128, C], BF16, name=f"c_{d}{k}",
                                          tag=f"c_{d}{k}") for k in range(KH)]
                    for k in range(KH):
                        nc.vector.memset(h16[d][k][:], 0.0)
                        nc.vector.memset(cst[d][k][:], 0.0)
                for k in range(KH):
                    nc.vector.memset(root_acc[ch][k][:], 0.0)

                for s in range(L):
                    for d in ("u", "d"):
                        t_src = s if d == "u" else L - 1 - s
                        nd = []
                        for k, (ko, ksz) in enumerate(KD_TILES):
                            t = nodep.tile([ksz, C], BF16, name=f"nd{k}",
                                           tag=f"nd{k}")
                            nc.sync.dma_start(
                                t[:], node_T[t_src, ko:ko + ksz, c0:c0 + C])
                            nd.append(t)
                        gates = []
                        for m in range(M12):
                            pm = pmm.tile([128, C], FP32, name="pm", tag="mm")
                            nk = KD if s == 0 else KD + KH
                            for k in range(KD):
                                nc.tensor.matmul(
                                    pm[:], w_t[d][k][:, m * 128:(m + 1) * 128],
                                    nd[k][:], start=(k == 0), stop=(k == nk - 1))
                            if s > 0:
                                for k in range(KH):
                                    nc.tensor.matmul(
                                        pm[:], u_t[d][k][:, m * 128:(m + 1) * 128],
                                        h16[d][k][:], start=False,
                                        stop=(k == KH - 1))
                            if debug and ch == 0 and s == 0 and d == "u" and m == 0:
                                pmc = gatep.tile([128, C], FP32, name="pmc",
                                                 tag="pmc", bufs=1)
                                nc.vector.tensor_copy(pmc[:], pm[:])
                                nc.sync.dma_start(dbg_pm[:, :], pmc[:])
                                for kk, (ko, ksz) in enumerate(KD_TILES):
                                    nc.sync.dma_start(dbg_nd[ko:ko + ksz, :],
                                                      nd[kk][:])
                            g = gatep.tile([128, C], BF16, name="g", tag="g")
                            func = (ACTF.Tanh if 2 * KH <= m < 3 * KH
                                    else ACTF.Sigmoid)
                            nc.scalar.activation(g[:], pm[:], func,
                                                 bias=b_t[d][m][:])
                            if debug and ch == 0 and s == 0 and d == "u":
                                nc.sync.dma_start(dbg_g0[m], g[:])
                            gates.append(g)
                        i_g, o_g, u_g, f_g = (gates[0:3], gates[3:6], gates[6:9],
                                              gates[9:12])
                        hnew = []
                        cnew = []
                        for k in range(KH):
                            tmp = gatep.tile([128, C], BF16, name="tmp", tag="g")
                            nc.vector.tensor_tensor(tmp[:], i_g[k][:], u_g[k][:],
                                                    ALU.mult)
                            cn = statep.tile([128, C], BF16, name=f"cn_{d}{k}",
                                             tag=f"c_{d}{k}")
                            if s == 0:
                                nc.vector.tensor_copy(cn[:], tmp[:])
                            else:
                                nc.vector.tensor_tensor(cn[:], f_g[k][:],
                                                        cst[d][k][:], ALU.mult)
                                nc.vector.tensor_tensor(cn[:], cn[:], tmp[:],
                                                        ALU.add)
                            tc_ = gatep.tile([128, C], BF16, name="tc", tag="g")
                            nc.scalar.activation(tc_[:], cn[:], ACTF.Tanh)
                            hn = statep.tile([128, C], BF16, name=f"hn_{d}{k}",
                                             tag=f"h_{d}{k}")
                            nc.vector.tensor_tensor(hn[:], o_g[k][:], tc_[:],
                                                    ALU.mult)
                            hnew.append(hn)
                            cnew.append(cn)
                        h16[d] = hnew
                        cst[d] = cnew
                        if debug and ch == 0 and s == 0 and d == "u":
                            for k in range(KH):
                                nc.sync.dma_start(dbg_h1[k], hnew[k][:])
                        if d == "u":
                            eq = eqp.tile([1, C], BF16, name="eq", tag="eq")
                            nc.vector.tensor_scalar(eq[:], root_t[:, c0:c0 + C],
                                                    float(s), None, ALU.is_equal)
                            mp = pmask.tile([128, C], FP32, name="mp", tag="mp")
                            nc.tensor.matmul(mp[:], ones_t[:], eq[:], start=True,
                                             stop=True)
                            # walrus requires an integer mask for CopyPredicated
                            mpi = eqp.tile([128, C], mybir.dt.uint8, name="mpi",
                                           tag="mpi", bufs=2)
                            nc.vector.tensor_copy(mpi[:], mp[:])
                            for k in range(KH):
                                nc.vector.copy_predicated(root_acc[ch][k][:],
                                                          mpi[:],
                                                          h16["u"][k][:])
                        else:
                            if s == 0:
                                for k in range(KH):
                                    end_t[ch][k] = capp.tile(
                                        [128, C], BF16, name=f"end{ch}_{k}",
                                        tag=f"end{ch}_{k}")
                                    nc.vector.tensor_copy(end_t[ch][k][:],
                                                          h16["d"][k][:])
                            if s == L - 1:
                                for k in range(KH):
                                    start_t[ch][k] = capp.tile(
                                        [128, C], BF16, name=f"start{ch}_{k}",
                                        tag=f"start{ch}_{k}")
                                    nc.vector.tensor_copy(start_t[ch][k][:],
                                                          h16["d"][k][:])

        if debug:
            for sp in range(2):
                for m in range(MT):
                    nc.sync.dma_start(dbg_span[sp, m], spanT[sp][m][:])
            for ch in range(NCH):
                for k in range(KH):
                    nc.sync.dma_start(dbg_racc[ch, k], root_acc[ch][k][:])
                    nc.sync.dma_start(dbg_start[ch, k], start_t[ch][k][:])
                    nc.sync.dma_start(dbg_end[ch, k], end_t[ch][k][:])

        # ---- phase 3: pair MLP -----------------------------------------
        with tc.tile_pool(name="mlpw", bufs=1) as mlpw, \
             tc.tile_pool(name="mlpp", bufs=4) as mlpp:
            w1_t = [loadc(mlpw, f"w1{k}", W1[k * 128:(k + 1) * 128, :],
                          [128, DEC_H], BF16) for k in range(K21)]
            w2_t = [loadc(mlpw, f"w2{k}", W2[k * 128:(k + 1) * 128, :],
                          [128, DEC_OUT], BF16) for k in range(M4)]
            for ch in range(NCH):
                c0 = ch * C
                feats = (root_acc[ch] + start_t[ch] + end_t[ch]
                         + [spanT[0][m][:, c0:c0 + C] for m in range(MT)]
                         + [spanT[1][m][:, c0:c0 + C] for m in range(MT)])
                z_t = []
                for m in range(M4):
                    zp = pmm.tile([128, C], FP32, name="zp2", tag="mm")
                    for k in range(K21):
                        fk = feats[k] if isinstance(feats[k], bass.AP) \
                            else feats[k][:]
                        nc.tensor.matmul(zp[:], w1_t[k][:, m * 128:(m + 1) * 128],
                                         fk, start=(k == 0), stop=(k == K21 - 1))
                    z = mlpp.tile([128, C], BF16, name="z", tag="z")
                    nc.scalar.activation(z[:], zp[:], ACTF.Tanh, bias=b1_t[m][:])
                    z_t.append(z)
                op = pout.tile([DEC_OUT, C], FP32, name="op", tag="op")
                for m in range(M4):
                    nc.tensor.matmul(op[:], w2_t[m][:], z_t[m][:], start=(m == 0),
                                     stop=(m == M4 - 1))
                osb = mlpp.tile([DEC_OUT, C], FP32, name="osb", tag="osb", bufs=2)
                nc.vector.tensor_scalar(osb[:], op[:], b2_t[:], None, ALU.add)
                nc.sync.dma_start(out_d[:, c0:c0 + C], osb[:])

    nc.compile()
    return nc


_CACHE = {}


def _get_program() -> bass.Bass:
    if "nc" not in _CACHE:
        _CACHE["nc"] = _build_program()
    return _CACHE["nc"]


def _prep_in_maps(inputs) -> list[dict]:
    f32 = np.float32
    node = np.asarray(inputs["node_embs"], f32)
    tokf = np.asarray(inputs["token_embs"], f32)
    rooti = np.asarray(inputs["root_idx"])
    # [P, L, D] -> per-core [L, D, PS] bf16
    node_sh = np.ascontiguousarray(
        node.reshape(NCORES, PS, L, D).transpose(0, 2, 3, 1)).astype(bf16)
    tok_sh = tokf.reshape(NCORES, NB, T, DT).astype(bf16)
    root_sh = rooti.reshape(NCORES, 1, PS).astype(f32)

    def span_arrays(st, ln):
        st = np.asarray(st).astype(f32)
        ln = np.asarray(ln).astype(f32)
        en = st + ln + 1.0
        rc = 1.0 / (ln + 1.0)
        return st, en, rc

    s1, e1, r1 = span_arrays(inputs["p1_st"], inputs["p1_len"])
    s2, e2, r2 = span_arrays(inputs["p2_st"], inputs["p2_len"])

    def pack_span(a1, a2):
        # [B, PB] x2 -> per-core [2, NB, JT, 128]
        a = np.stack([a1, a2])  # [2, B, PB]
        a = a.reshape(2, NCORES, NB, JT, 128).transpose(1, 0, 2, 3, 4)
        return np.ascontiguousarray(a.astype(f32))

    stp, enp, rcp = pack_span(s1, s2), pack_span(e1, e2), pack_span(r1, r2)
    zp = np.zeros_like(stp)
    # [NCORES, 2, NB, JT, 128, 4]: st | en | recip | pad
    sp_all = np.ascontiguousarray(np.stack([stp, enp, rcp, zp], axis=-1))

    Wu_h = np.concatenate([np.asarray(inputs["Wiou_u"], f32),
                           np.asarray(inputs["Wf_u"], f32)], axis=1).astype(bf16)
    Wd_h = np.concatenate([np.asarray(inputs["Wiou_d"], f32),
                           np.asarray(inputs["Wf_d"], f32)], axis=1).astype(bf16)
    Uu_h = np.concatenate([np.asarray(inputs["Uiou_u"], f32),
                           np.asarray(inputs["Uf_u"], f32)], axis=1).astype(bf16)
    Ud_h = np.concatenate([np.asarray(inputs["Uiou_d"], f32),
                           np.asarray(inputs["Uf_d"], f32)], axis=1).astype(bf16)
    bu_h = np.concatenate([np.asarray(inputs["biou_u"], f32),
                           np.asarray(inputs["bf_u"], f32)]).reshape(M12, 128, 1)
    bd_h = np.concatenate([np.asarray(inputs["biou_d"], f32),
                           np.asarray(inputs["bf_d"], f32)]).reshape(M12, 128, 1)
    W1_h = np.asarray(inputs["W1"], f32).astype(bf16)
    W2_h = np.asarray(inputs["W2"], f32).astype(bf16)
    b1_h = np.asarray(inputs["b1"], f32).reshape(M4, 128, 1)
    b2_h = np.asarray(inputs["b2"], f32).reshape(DEC_OUT, 1)
    ones_h = np.ones((1, 128), bf16)
    iota_h = np.broadcast_to(np.arange(T, dtype=f32), (128, T)).copy()

    in_maps = []
    for c in range(NCORES):
        in_maps.append({
            "node_T": node_sh[c], "tok": tok_sh[c], "root": root_sh[c],
            "sp_all": sp_all[c],
            "Wu": Wu_h, "Wd": Wd_h, "Uu": Uu_h, "Ud": Ud_h,
            "W1": W1_h, "W2": W2_h, "bu": bu_h, "bd": bd_h,
            "b1": b1_h, "b2": b2_h, "ones": ones_h, "iota_d": iota_h,
        })
    return in_maps


def run(inputs, **kwargs):
    """Run on hardware; returns (output [P, DEC_OUT] fp32, BassKernelResults)."""
    nc = _get_program()
    in_maps = _prep_in_maps(inputs)
    res = run_bass_kernel_spmd(nc, in_maps, list(range(NCORES)), **kwargs)
    outs = [np.asarray(r["out"], np.float32).T for r in res.results]  # [PS, 7] each
    return np.concatenate(outs, axis=0), res


def kernel(**inputs) -> np.ndarray:
    out, _ = run(inputs)
    return out


# revision 28
# speedup vs baseline: 33.7107x; 33.7107x over previous
"""Trainium2 Bass kernel for nn_DepPairingLayer (bidirectional chain-TreeLSTM over
shortest-path node chains + span mean-pooling + pair MLP), SPMD across 8 NeuronCores.

Sharding: data-parallel over the pair dimension P=8192 (1024 pairs/core, which is
exactly 4 batches x 256 pairs for the span pooling); all weights replicated.

Device layout is feature-major: activations live as [features(partitions), pairs(free)]
so the LSTM recurrence h @ U and the input projection x @ W become matmuls with the
weights as the stationary operand ([in_dim, out_dim] blocks) and the data as the
moving operand. All matmuls run in bf16 with fp32 PSUM accumulation (measured
end-to-end rel-absmax error vs the fp32 reference: ~4e-3).

Host-side prep (layout/cast only): node_embs is pre-transposed to [L, D, pairs] and
cast to bf16 so per-step slices DMA as dense [128, C] tiles; weights are concatenated
([Wiou|Wf] -> [D, 4H]) and cast; span start/end/recip scalars are laid out
partition-major.
"""

from contextlib import ExitStack

import numpy as np
import ml_dtypes

import concourse.bass as bass
import concourse.mybir as mybir
import concourse.tile as tile
from concourse import bacc
from concourse.bass_utils import run_bass_kernel_spmd
from concourse.masks import make_identity

bf16 = ml_dtypes.bfloat16
FP32 = mybir.dt.float32
BF16 = mybir.dt.bfloat16
ALU = mybir.AluOpType
ACTF = mybir.ActivationFunctionType

# problem dims (hardcoded per contract)
NCORES = 8
B, PB, L, D, H, DT, T = 32, 256, 16, 832, 384, 768, 512
P = B * PB                      # 8192 pairs
PS = P // NCORES                # 1024 pairs per core
NB = B // NCORES                # 4 batches per core
C = 512                         # pair-chunk (matmul moving free dim)
NCH = PS // C                   # 2 chunks per core
H4 = 4 * H                      # 1536 = i|o|u|f
# D=832 is NOT a multiple of 128: 6 full k-tiles + one 64-row tile
KD_TILES = [(i * 128, 128) for i in range(D // 128)] + (
    [(D - D % 128, D % 128)] if D % 128 else [])
KD = len(KD_TILES)              # 7 k-tiles of node features
M12 = H4 // 128                 # 12 m-tiles of gate features
KH = H // 128                   # 3 k-tiles of hidden
DEC_IN, DEC_H, DEC_OUT = 3 * H + 2 * DT, 512, 7
K21 = DEC_IN // 128             # 21 feature k-tiles for W1
M4 = DEC_H // 128               # 4 m-tiles for W1 output
MT = DT // 128                  # 6 span-feature m-tiles
JT = PB // 128                  # 2 pair-tiles per batch (for masks)


def _build_program(debug: bool = False, loop_n: int = 0) -> bass.Bass:
    """loop_n > 0 wraps the whole body in a For_i loop executing it loop_n
    times (identical work each iteration) — used only for timing via
    (T(N) - T(1)) / (N - 1)."""
    nc = bacc.Bacc("TRN2", target_bir_lowering=False, debug=False,
                   num_devices=NCORES)
    dp = nc.declare_dram_parameter
    if debug:
        dbg_span = dp("dbg_span", [2, MT, 128, PS], BF16, isOutput=True)
        dbg_racc = dp("dbg_racc", [NCH, KH, 128, C], BF16, isOutput=True)
        dbg_start = dp("dbg_start", [NCH, KH, 128, C], BF16, isOutput=True)
        dbg_end = dp("dbg_end", [NCH, KH, 128, C], BF16, isOutput=True)
        dbg_g0 = dp("dbg_g0", [M12, 128, C], BF16, isOutput=True)
        dbg_h1 = dp("dbg_h1", [KH, 128, C], BF16, isOutput=True)
        dbg_nd = dp("dbg_nd", [D, C], BF16, isOutput=True)
        dbg_pm = dp("dbg_pm", [128, C], FP32, isOutput=True)

    node_T = dp("node_T", [L, D, PS], BF16, isOutput=False)
    tok = dp("tok", [NB, T, DT], BF16, isOutput=False)
    root = dp("root", [1, PS], FP32, isOutput=False)
    sp_all = dp("sp_all", [2, NB, JT, 128, 4], FP32, isOutput=False)
    Wu = dp("Wu", [D, H4], BF16, isOutput=False)
    Wd = dp("Wd", [D, H4], BF16, isOutput=False)
    Uu = dp("Uu", [H, H4], BF16, isOutput=False)
    Ud = dp("Ud", [H, H4], BF16, isOutput=False)
    W1 = dp("W1", [DEC_IN, DEC_H], BF16, isOutput=False)
    W2 = dp("W2", [DEC_H, DEC_OUT], BF16, isOutput=False)
    bu = dp("bu", [M12, 128, 1], FP32, isOutput=False)
    bd = dp("bd", [M12, 128, 1], FP32, isOutput=False)
    b1 = dp("b1", [M4, 128, 1], FP32, isOutput=False)
    b2 = dp("b2", [DEC_OUT, 1], FP32, isOutput=False)
    ones = dp("ones", [1, 128], BF16, isOutput=False)
    iota_d = dp("iota_d", [128, T], FP32, isOutput=False)
    out_d = dp("out", [DEC_OUT, PS], FP32, isOutput=True)

    def loadc(pool, name, src_ap, shape, dtype, bufs=1):
        t = pool.tile(shape, dtype, name=name, tag=name, bufs=bufs)
        nc.sync.dma_start(t[:], src_ap)
        return t

    with tile.TileContext(nc) as tc, ExitStack() as ctx:
        if loop_n:
            ctx.enter_context(tc.For_i(0, loop_n, 1))
        # whole-program pools
        cpool = ctx.enter_context(tc.tile_pool(name="const", bufs=1))
        spanp = ctx.enter_context(tc.tile_pool(name="spanp", bufs=1))
        capp = ctx.enter_context(tc.tile_pool(name="capp", bufs=1))
        pmm = ctx.enter_context(tc.tile_pool(name="pmm", bufs=4, space="PSUM"))
        ptp = ctx.enter_context(tc.tile_pool(name="ptp", bufs=2, space="PSUM"))
        pmask = ctx.enter_context(tc.tile_pool(name="pmask", bufs=1, space="PSUM"))
        pout = ctx.enter_context(tc.tile_pool(name="pout", bufs=1, space="PSUM"))

        bu_t = [loadc(cpool, f"bu{m}", bu[m], [128, 1], FP32) for m in range(M12)]
        bd_t = [loadc(cpool, f"bd{m}", bd[m], [128, 1], FP32) for m in range(M12)]
        b1_t = [loadc(cpool, f"b1{m}", b1[m], [128, 1], FP32) for m in range(M4)]
        b2_t = loadc(cpool, "b2t", b2[:, :], [DEC_OUT, 1], FP32)
        ones_t = loadc(cpool, "onest", ones[:, :], [1, 128], BF16)
        root_t = loadc(cpool, "roott", root[:, :], [1, PS], FP32)
        iota_t = loadc(cpool, "iota", iota_d[:, :], [128, T], FP32)
        ident = cpool.tile([128, 128], BF16, name="ident", tag="ident")
        make_identity(nc, ident[:])

        b_t = {"u": bu_t, "d": bd_t}

        # spanT[sp][m]: [128, PS] bf16 feature-major span means (whole program)
        spanT = [[spanp.tile([128, PS], BF16, name=f"span{sp}_{m}",
                             tag=f"span{sp}_{m}") for m in range(MT)]
                 for sp in range(2)]
        # per-chunk LSTM summary tiles (whole program; consumed by the MLP phase)
        root_acc = [[capp.tile([128, C], BF16, name=f"racc{ch}_{k}",
                               tag=f"racc{ch}_{k}") for k in range(KH)]
                    for ch in range(NCH)]
        start_t = [[None] * KH for _ in range(NCH)]
        end_t = [[None] * KH for _ in range(NCH)]

        # ---- phase 1: span mean pooling --------------------------------
        with tc.tile_pool(name="tokp", bufs=2) as tokp, \
             tc.tile_pool(name="mwork", bufs=2) as mwork:
            for b in range(NB):
                tk = []
                for tb in range(T // 128):
                    t = tokp.tile([128, DT], BF16, name=f"tok{tb}", tag=f"tok{tb}")
                    nc.sync.dma_start(t[:], tok[b, tb * 128:(tb + 1) * 128, :])
                    tk.append(t)
                for sp in range(2):
                    maskT = [mwork.tile([128, PB], BF16, name=f"mT{tb}",
                                        tag=f"mT{tb}") for tb in range(T // 128)]
                    for jt in range(JT):
                        sc3 = mwork.tile([128, 4], FP32, name="sc3", tag="sc3",
                                         bufs=4)
                        nc.sync.dma_start(sc3[:], sp_all[sp, b, jt])
                        cmp1 = mwork.tile([128, T], BF16, name="cmp1", tag="cmp1")
                        cmp2 = mwork.tile([128, T], BF16, name="cmp2", tag="cmp2")
                        nc.vector.tensor_scalar(cmp1[:], iota_t[:], sc3[:, 0:1],
                                                None, ALU.is_ge)
                        nc.vector.tensor_scalar(cmp2[:], iota_t[:], sc3[:, 1:2],
                                                None, ALU.is_lt)
                        m16 = mwork.tile([128, T], BF16, name="m16", tag="m16")
                        nc.vector.scalar_tensor_tensor(m16[:], cmp1[:], sc3[:, 2:3],
                                                       cmp2[:], op0=ALU.mult,
                                                       op1=ALU.mult)
                        for tb in range(T // 128):
                            tp = ptp.tile([128, 128], BF16, name="tp", tag="tp")
                            nc.tensor.transpose(
                                tp[:], m16[:, tb * 128:(tb + 1) * 128], ident[:])
                            nc.vector.tensor_copy(
                                maskT[tb][:, jt * 128:(jt + 1) * 128], tp[:])
                    for m in range(MT):
                        zp = pmm.tile([128, PB], FP32, name="zp", tag="mm")
                        for tb in range(T // 128):
                            nc.tensor.matmul(zp[:], tk[tb][:, m * 128:(m + 1) * 128],
                                             maskT[tb][:], start=(tb == 0),
                                             stop=(tb == T // 128 - 1))
                        nc.vector.tensor_copy(spanT[sp][m][:, b * PB:(b + 1) * PB],
                                              zp[:])

        # ---- phase 2: bidirectional chain-LSTM per pair-chunk ----------
        with tc.tile_pool(name="lstmw", bufs=1) as lstmw, \
             tc.tile_pool(name="nodep", bufs=3) as nodep, \
             tc.tile_pool(name="statep", bufs=2) as statep, \
             tc.tile_pool(name="gatep", bufs=16) as gatep, \
             tc.tile_pool(name="eqp", bufs=4) as eqp:
            wu_t = [loadc(lstmw, f"wu{k}", Wu[ko:ko + ksz, :], [ksz, H4], BF16)
                    for k, (ko, ksz) in enumerate(KD_TILES)]
            wd_t = [loadc(lstmw, f"wd{k}", Wd[ko:ko + ksz, :], [ksz, H4], BF16)
                    for k, (ko, ksz) in enumerate(KD_TILES)]
            uu_t = [loadc(lstmw, f"uu{k}", Uu[k * 128:(k + 1) * 128, :], [128, H4],
                          BF16) for k in range(KH)]
            ud_t = [loadc(lstmw, f"ud{k}", Ud[k * 128:(k + 1) * 128, :], [128, H4],
                          BF16) for k in range(KH)]
            w_t = {"u": wu_t, "d": wd_t}
            u_t = {"u": uu_t, "d": ud_t}

            for ch in range(NCH):
                c0 = ch * C
                h16 = {}
                cst = {}
                for d in ("u", "d"):
                    h16[d] = [statep.tile([128, C], BF16, name=f"h_{d}{k}",
                                          tag=f"h_{d}{k}") for k in range(KH)]
                    cst[d] = [statep.tile([128, C], BF16, name=f"c_{d}{k}",
                                          tag=f"c_{d}{k}") for k in range(KH)]
                    for k in range(KH):
                        nc.vector.memset(h16[d][k][:], 0.0)
                        nc.vector.memset(cst[d][k][:], 0.0)
                for k in range(KH):
                    nc.vector.memset(root_acc[ch][k][:], 0.0)

                for s in range(L):
                    for d in ("u", "d"):
                        t_src = s if d == "u" else L - 1 - s
                        nd = []
                        for k, (ko, ksz) in enumerate(KD_TILES):
                            t = nodep.tile([ksz, C], BF16, name=f"nd{k}",
                                           tag=f"nd{k}")
                            nc.sync.dma_start(
                                t[:], node_T[t_src, ko:ko + ksz, c0:c0 + C])
                            nd.append(t)
                        gates = []
                        for m in range(M12):
                            pm = pmm.tile([128, C], FP32, name="pm", tag="mm")
                            nk = KD if s == 0 else KD + KH
                            for k in range(KD):
                                nc.tensor.matmul(
                                    pm[:], w_t[d][k][:, m * 128:(m + 1) * 128],
                                    nd[k][:], start=(k == 0), stop=(k == nk - 1))
                            if s > 0:
                                for k in range(KH):
                                    nc.tensor.matmul(
                                        pm[:], u_t[d][k][:, m * 128:(m + 1) * 128],
                                        h16[d][k][:], start=False,
                                        stop=(k == KH - 1))
                            if debug and ch == 0 and s == 0 and d == "u" and m == 0:
                                pmc = gatep.tile([128, C], FP32, name="pmc",
                                                 tag="pmc", bufs=1)
                                nc.vector.tensor_copy(pmc[:], pm[:])
                                nc.sync.dma_start(dbg_pm[:, :], pmc[:])
                                for kk, (ko, ksz) in enumerate(KD_TILES):
                                    nc.sync.dma_start(dbg_nd[ko:ko + ksz, :],
                                                      nd[kk][:])
                            g = gatep.tile([128, C], BF16, name="g", tag="g")
                            func = (ACTF.Tanh if 2 * KH <= m < 3 * KH
                                    else ACTF.Sigmoid)
                            nc.scalar.activation(g[:], pm[:], func,
                                                 bias=b_t[d][m][:])
                            if debug and ch == 0 and s == 0 and d == "u":
                                nc.sync.dma_start(dbg_g0[m], g[:])
                            gates.append(g)
                        i_g, o_g, u_g, f_g = (gates[0:3], gates[3:6], gates[6:9],
                                              gates[9:12])
                        hnew = []
                        cnew = []
                        for k in range(KH):
                            tmp = gatep.tile([128, C], BF16, name="tmp", tag="g")
                            nc.vector.tensor_tensor(tmp[:], i_g[k][:], u_g[k][:],
                                                    ALU.mult)
                            cn = statep.tile([128, C], BF16, name=f"cn_{d}{k}",
                                             tag=f"c_{d}{k}")
                            if s == 0:
                                nc.vector.tensor_copy(cn[:], tmp[:])
                            else:
                                nc.vector.tensor_tensor(cn[:], f_g[k][:],
                                                        cst[d][k][:], ALU.mult)
                                nc.vector.tensor_tensor(cn[:], cn[:], tmp[:],
                                                        ALU.add)
                            tc_ = gatep.tile([128, C], BF16, name="tc", tag="g")
                            nc.scalar.activation(tc_[:], cn[:], ACTF.Tanh)
                            hn = statep.tile([128, C], BF16, name=f"hn_{d}{k}",
                                             tag=f"h_{d}{k}")
                            nc.vector.tensor_tensor(hn[:], o_g[k][:], tc_[:],
                                                    ALU.mult)
                            hnew.append(hn)
                            cnew.append(cn)
                        h16[d] = hnew
                        cst[d] = cnew
                        if debug and ch == 0 and s == 0 and d == "u":
                            for k in range(KH):
                                nc.sync.dma_start(dbg_h1[k], hnew[k][:])
                        if d == "u":
                            eq = eqp.tile([1, C], BF16, name="eq", tag="eq")
                            nc.vector.tensor_scalar(eq[:], root_t[:, c0:c0 + C],
                                                    float(s), None, ALU.is_equal)
                            mp = pmask.tile([128, C], FP32, name="mp", tag="mp")
                            nc.tensor.matmul(mp[:], ones_t[:], eq[:], start=True,
                                             stop=True)
                            # walrus requires an integer mask for CopyPredicated
                            mpi = eqp.tile([128, C], mybir.dt.uint8, name="mpi",
                                           tag="mpi", bufs=2)
                            nc.vector.tensor_copy(mpi[:], mp[:])
                            for k in range(KH):
                                nc.vector.copy_predicated(root_acc[ch][k][:],
                                                          mpi[:],
                                                          h16["u"][k][:])
                        else:
                            if s == 0:
                                for k in range(KH):
                                    end_t[ch][k] = capp.tile(
                                        [128, C], BF16, name=f"end{ch}_{k}",
                                        tag=f"end{ch}_{k}")
                                    nc.vector.tensor_copy(end_t[ch][k][:],
                                                          h16["d"][k][:])
                            if s == L - 1:
                                for k in range(KH):
                                    start_t[ch][k] = capp.tile(
                                        [128, C], BF16, name=f"start{ch}_{k}",
                                        tag=f"start{ch}_{k}")
                                    nc.vector.tensor_copy(start_t[ch][k][:],
                                                          h16["d"][k][:])

        if debug:
            for sp in range(2):
                for m in range(MT):
                    nc.sync.dma_start(dbg_span[sp, m], spanT[sp][m][:])
            for ch in range(NCH):
                for k in range(KH):
                    nc.sync.dma_start(dbg_racc[ch, k], root_acc[ch][k][:])
                    nc.sync.dma_start(dbg_start[ch, k], start_t[ch][k][:])
                    nc.sync.dma_start(dbg_end[ch, k], end_t[ch][k][:])

        # ---- phase 3: pair MLP -----------------------------------------
        with tc.tile_pool(name="mlpw", bufs=1) as mlpw, \
             tc.tile_pool(name="mlpp", bufs=4) as mlpp:
            w1_t = [loadc(mlpw, f"w1{k}", W1[k * 128:(k + 1) * 128, :],
                          [128, DEC_H], BF16) for k in range(K21)]
            w2_t = [loadc(mlpw, f"w2{k}", W2[k * 128:(k + 1) * 128, :],
                          [128, DEC_OUT], BF16) for k in range(M4)]
            for ch in range(NCH):
                c0 = ch * C
                feats = (root_acc[ch] + start_t[ch] + end_t[ch]
                         + [spanT[0][m][:, c0:c0 + C] for m in range(MT)]
                         + [spanT[1][m][:, c0:c0 + C] for m in range(MT)])
                z_t = []
                for m in range(M4):
                    zp = pmm.tile([128, C], FP32, name="zp2", tag="mm")
                    for k in range(K21):
                        fk = feats[k] if isinstance(feats[k], bass.AP) \
                            else feats[k][:]
                        nc.tensor.matmul(zp[:], w1_t[k][:, m * 128:(m + 1) * 128],
                                         fk, start=(k == 0), stop=(k == K21 - 1))
                    z = mlpp.tile([128, C], BF16, name="z", tag="z")
                    nc.scalar.activation(z[:], zp[:], ACTF.Tanh, bias=b1_t[m][:])
                    z_t.append(z)
                op = pout.tile([DEC_OUT, C], FP32, name="op", tag="op")
                for m in range(M4):
                    nc.tensor.matmul(op[:], w2_t[m][:], z_t[m][:], start=(m == 0),
                                     stop=(m == M4 - 1))
                osb = mlpp.tile([DEC_OUT, C], FP32, name="osb", tag="osb", bufs=2)
                nc.vector.tensor_scalar(osb[:], op[:], b2_t[:], None, ALU.add)
                nc.sync.dma_start(out_d[:, c0:c0 + C], osb[:])

    nc.compile()
    return nc


_CACHE = {}


def _get_program() -> bass.Bass:
    if "nc" not in _CACHE:
        _CACHE["nc"] = _build_program()
    return _CACHE["nc"]


def _prep_in_maps(inputs) -> list[dict]:
    f32 = np.float32
    node = np.asarray(inputs["node_embs"], f32)
    tokf = np.asarray(inputs["token_embs"], f32)
    rooti = np.asarray(inputs["root_idx"])
    # [P, L, D] -> per-core [L, D, PS] bf16
    node_sh = np.ascontiguousarray(
        node.reshape(NCORES, PS, L, D).transpose(0, 2, 3, 1)).astype(bf16)
    tok_sh = tokf.reshape(NCORES, NB, T, DT).astype(bf16)
    root_sh = rooti.reshape(NCORES, 1, PS).astype(f32)

    def span_arrays(st, ln):
        st = np.asarray(st).astype(f32)
        ln = np.asarray(ln).astype(f32)
        en = st + ln + 1.0
        rc = 1.0 / (ln + 1.0)
        return st, en, rc

    s1, e1, r1 = span_arrays(inputs["p1_st"], inputs["p1_len"])
    s2, e2, r2 = span_arrays(inputs["p2_st"], inputs["p2_len"])

    def pack_span(a1, a2):
        # [B, PB] x2 -> per-core [2, NB, JT, 128]
        a = np.stack([a1, a2])  # [2, B, PB]
        a = a.reshape(2, NCORES, NB, JT, 128).transpose(1, 0, 2, 3, 4)
        return np.ascontiguousarray(a.astype(f32))

    stp, enp, rcp = pack_span(s1, s2), pack_span(e1, e2), pack_span(r1, r2)
    zp = np.zeros_like(stp)
    # [NCORES, 2, NB, JT, 128, 4]: st | en | recip | pad
    sp_all = np.ascontiguousarray(np.stack([stp, enp, rcp, zp], axis=-1))

    Wu_h = np.concatenate([np.asarray(inputs["Wiou_u"], f32),
                           np.asarray(inputs["Wf_u"], f32)], axis=1).astype(bf16)
    Wd_h = np.concatenate([np.asarray(inputs["Wiou_d"], f32),
                           np.asarray(inputs["Wf_d"], f32)], axis=1).astype(bf16)
    Uu_h = np.concatenate([np.asarray(inputs["Uiou_u"], f32),
                           np.asarray(inputs["Uf_u"], f32)], axis=1).astype(bf16)
    Ud_h = np.concatenate([np.asarray(inputs["Uiou_d"], f32),
                           np.asarray(inputs["Uf_d"], f32)], axis=1).astype(bf16)
    bu_h = np.concatenate([np.asarray(inputs["biou_u"], f32),
                           np.asarray(inputs["bf_u"], f32)]).reshape(M12, 128, 1)
    bd_h = np.concatenate([np.asarray(inputs["biou_d"], f32),
                           np.asarray(inputs["bf_d"], f32)]).reshape(M12, 128, 1)
    W1_h = np.asarray(inputs["W1"], f32).astype(bf16)
    W2_h = np.asarray(inputs["W2"], f32).astype(bf16)
    b1_h = np.asarray(inputs["b1"], f32).reshape(M4, 128, 1)
    b2_h = np.asarray(inputs["b2"], f32).reshape(DEC_OUT, 1)
    ones_h = np.ones((1, 128), bf16)
    iota_h = np.broadcast_to(np.arange(T, dtype=f32), (128, T)).copy()

    in_maps = []
    for c in range(NCORES):
        in_maps.append({
            "node_T": node_sh[c], "tok": tok_sh[c], "root": root_sh[c],
            "sp_all": sp_all[c],
            "Wu": Wu_h, "Wd": Wd_h, "Uu": Uu_h, "Ud": Ud_h,
            "W1": W1_h, "W2": W2_h, "bu": bu_h, "bd": bd_h,
            "b1": b1_h, "b2": b2_h, "ones": ones_h, "iota_d": iota_h,
        })
    return in_maps


def run(inputs, **kwargs):
    """Run on hardware; returns (output [P, DEC_OUT] fp32, BassKernelResults)."""
    nc = _get_program()
    in_maps = _prep_in_maps(inputs)
    res = run_bass_kernel_spmd(nc, in_maps, list(range(NCORES)), **kwargs)
    outs = [np.asarray(r["out"], np.float32).T for r in res.results]  # [PS, 7] each
    return np.concatenate(outs, axis=0), res


def kernel(**inputs) -> np.ndarray:
    out, _ = run(inputs)
    return out


# revision 31
# speedup vs baseline: 40.5788x; 1.2037x over previous
"""Trainium2 Bass kernel for nn_DepPairingLayer (bidirectional chain-TreeLSTM over
shortest-path node chains + span mean-pooling + pair MLP), SPMD across 8 NeuronCores.

Sharding: data-parallel over the pair dimension P=8192 (1024 pairs/core, which is
exactly 4 batches x 256 pairs for the span pooling); all weights replicated.

Device layout is feature-major: activations live as [features(partitions), pairs(free)]
so the LSTM recurrence h @ U and the input projection x @ W become matmuls with the
weights as the stationary operand ([in_dim, out_dim] blocks) and the data as the
moving operand. All matmuls run in bf16 with fp32 PSUM accumulation (measured
end-to-end rel-absmax error vs the fp32 reference: ~4e-3).

Host-side prep (layout/cast only): node_embs is pre-transposed to [L, D, pairs] and
cast to bf16 so per-step slices DMA as dense [128, C] tiles; weights are concatenated
([Wiou|Wf] -> [D, 4H]) and cast; span start/end/recip scalars are laid out
partition-major.
"""

from contextlib import ExitStack

import numpy as np
import ml_dtypes

import concourse.bass as bass
import concourse.mybir as mybir
import concourse.tile as tile
from concourse import bacc
from concourse.bass_utils import run_bass_kernel_spmd
from concourse.masks import make_identity

bf16 = ml_dtypes.bfloat16
FP32 = mybir.dt.float32
BF16 = mybir.dt.bfloat16
ALU = mybir.AluOpType
ACTF = mybir.ActivationFunctionType

# problem dims (hardcoded per contract)
NCORES = 8
B, PB, L, D, H, DT, T = 32, 256, 16, 832, 384, 768, 512
P = B * PB                      # 8192 pairs
PS = P // NCORES                # 1024 pairs per core
NB = B // NCORES                # 4 batches per core
C = 512                         # pair-chunk (matmul moving free dim)
NCH = PS // C                   # 2 chunks per core
H4 = 4 * H                      # 1536 = i|o|u|f
# D=832 is NOT a multiple of 128: 6 full k-tiles + one 64-row tile
KD_TILES = [(i * 128, 128) for i in range(D // 128)] + (
    [(D - D % 128, D % 128)] if D % 128 else [])
KD = len(KD_TILES)              # 7 k-tiles of node features
M12 = H4 // 128                 # 12 m-tiles of gate features
KH = H // 128                   # 3 k-tiles of hidden
DEC_IN, DEC_H, DEC_OUT = 3 * H + 2 * DT, 512, 7
K21 = DEC_IN // 128             # 21 feature k-tiles for W1
M4 = DEC_H // 128               # 4 m-tiles for W1 output
MT = DT // 128                  # 6 span-feature m-tiles
JT = PB // 128                  # 2 pair-tiles per batch (for masks)


def _build_program(debug: bool = False, loop_n: int = 0,
                   probe: str = "") -> bass.Bass:
    """loop_n > 0 wraps the whole body in a For_i loop executing it loop_n
    times (identical work each iteration) — used only for timing via
    (T(N) - T(1)) / (N - 1)."""
    nc = bacc.Bacc("TRN2", target_bir_lowering=False, debug=False,
                   num_devices=NCORES)
    dp = nc.declare_dram_parameter
    if debug:
        dbg_span = dp("dbg_span", [2, MT, 128, PS], BF16, isOutput=True)
        dbg_racc = dp("dbg_racc", [NCH, KH, 128, C], BF16, isOutput=True)
        dbg_start = dp("dbg_start", [NCH, KH, 128, C], BF16, isOutput=True)
        dbg_end = dp("dbg_end", [NCH, KH, 128, C], BF16, isOutput=True)
        dbg_g0 = dp("dbg_g0", [M12, 128, C], BF16, isOutput=True)
        dbg_h1 = dp("dbg_h1", [KH, 128, C], BF16, isOutput=True)
        dbg_nd = dp("dbg_nd", [D, C], BF16, isOutput=True)
        dbg_pm = dp("dbg_pm", [128, C], FP32, isOutput=True)

    node_T = dp("node_T", [L, D, PS], BF16, isOutput=False)
    tok = dp("tok", [NB, T, DT], BF16, isOutput=False)
    root = dp("root", [1, PS], FP32, isOutput=False)
    sp_all = dp("sp_all", [2, NB, JT, 128, 4], FP32, isOutput=False)
    Wu = dp("Wu", [D, H4], BF16, isOutput=False)
    Wd = dp("Wd", [D, H4], BF16, isOutput=False)
    Uu = dp("Uu", [H, H4], BF16, isOutput=False)
    Ud = dp("Ud", [H, H4], BF16, isOutput=False)
    W1 = dp("W1", [DEC_IN, DEC_H], BF16, isOutput=False)
    W2 = dp("W2", [DEC_H, DEC_OUT], BF16, isOutput=False)
    bu = dp("bu", [M12, 128, 1], FP32, isOutput=False)
    bd = dp("bd", [M12, 128, 1], FP32, isOutput=False)
    b1 = dp("b1", [M4, 128, 1], FP32, isOutput=False)
    b2 = dp("b2", [DEC_OUT, 1], FP32, isOutput=False)
    ones = dp("ones", [1, 128], BF16, isOutput=False)
    iota_d = dp("iota_d", [128, T], FP32, isOutput=False)
    out_d = dp("out", [DEC_OUT, PS], FP32, isOutput=True)

    def loadc(pool, name, src_ap, shape, dtype, bufs=1):
        t = pool.tile(shape, dtype, name=name, tag=name, bufs=bufs)
        nc.sync.dma_start(t[:], src_ap)
        return t

    with tile.TileContext(nc) as tc, ExitStack() as ctx:
        if loop_n:
            ctx.enter_context(tc.For_i(0, loop_n, 1))
        # whole-program pools
        cpool = ctx.enter_context(tc.tile_pool(name="const", bufs=1))
        spanp = ctx.enter_context(tc.tile_pool(name="spanp", bufs=1))
        capp = ctx.enter_context(tc.tile_pool(name="capp", bufs=1))
        pmm = ctx.enter_context(tc.tile_pool(name="pmm", bufs=5, space="PSUM"))
        ptp = ctx.enter_context(tc.tile_pool(name="ptp", bufs=1, space="PSUM"))
        pmask = ctx.enter_context(tc.tile_pool(name="pmask", bufs=1, space="PSUM"))
        pout = ctx.enter_context(tc.tile_pool(name="pout", bufs=1, space="PSUM"))

        bu_t = [loadc(cpool, f"bu{m}", bu[m], [128, 1], FP32) for m in range(M12)]
        bd_t = [loadc(cpool, f"bd{m}", bd[m], [128, 1], FP32) for m in range(M12)]
        b1_t = [loadc(cpool, f"b1{m}", b1[m], [128, 1], FP32) for m in range(M4)]
        b2_t = loadc(cpool, "b2t", b2[:, :], [DEC_OUT, 1], FP32)
        ones_t = loadc(cpool, "onest", ones[:, :], [1, 128], BF16)
        root_t = loadc(cpool, "roott", root[:, :], [1, PS], FP32)
        iota_t = loadc(cpool, "iota", iota_d[:, :], [128, T], FP32)
        ident = cpool.tile([128, 128], BF16, name="ident", tag="ident")
        make_identity(nc, ident[:])

        b_t = {"u": bu_t, "d": bd_t}

        # spanT[sp][m]: [128, PS] bf16 feature-major span means (whole program)
        spanT = [[spanp.tile([128, PS], BF16, name=f"span{sp}_{m}",
                             tag=f"span{sp}_{m}") for m in range(MT)]
                 for sp in range(2)]
        # per-chunk LSTM summary tiles (whole program; consumed by the MLP phase)
        root_acc = [[capp.tile([128, C], BF16, name=f"racc{ch}_{k}",
                               tag=f"racc{ch}_{k}") for k in range(KH)]
                    for ch in range(NCH)]
        start_t = [[None] * KH for _ in range(NCH)]
        end_t = [[None] * KH for _ in range(NCH)]

        # ---- phase 1: span mean pooling --------------------------------
        with tc.tile_pool(name="tokp", bufs=2) as tokp, \
             tc.tile_pool(name="mwork", bufs=2) as mwork:
            for b in range(NB):
                tk = []
                for tb in range(T // 128):
                    t = tokp.tile([128, DT], BF16, name=f"tok{tb}", tag=f"tok{tb}")
                    nc.sync.dma_start(t[:], tok[b, tb * 128:(tb + 1) * 128, :])
                    tk.append(t)
                for sp in range(2):
                    maskT = [mwork.tile([128, PB], BF16, name=f"mT{tb}",
                                        tag=f"mT{tb}") for tb in range(T // 128)]
                    for jt in range(JT):
                        sc3 = mwork.tile([128, 4], FP32, name="sc3", tag="sc3",
                                         bufs=4)
                        nc.sync.dma_start(sc3[:], sp_all[sp, b, jt])
                        cmp1 = mwork.tile([128, T], BF16, name="cmp1", tag="cmp1")
                        cmp2 = mwork.tile([128, T], BF16, name="cmp2", tag="cmp2")
                        nc.vector.tensor_scalar(cmp1[:], iota_t[:], sc3[:, 0:1],
                                                None, ALU.is_ge)
                        nc.vector.tensor_scalar(cmp2[:], iota_t[:], sc3[:, 1:2],
                                                None, ALU.is_lt)
                        m16 = mwork.tile([128, T], BF16, name="m16", tag="m16")
                        nc.vector.scalar_tensor_tensor(m16[:], cmp1[:], sc3[:, 2:3],
                                                       cmp2[:], op0=ALU.mult,
                                                       op1=ALU.mult)
                        for tb in range(T // 128):
                            tp = ptp.tile([128, 128], BF16, name="tp", tag="tp")
                            nc.tensor.transpose(
                                tp[:], m16[:, tb * 128:(tb + 1) * 128], ident[:])
                            nc.vector.tensor_copy(
                                maskT[tb][:, jt * 128:(jt + 1) * 128], tp[:])
                    for m in range(MT):
                        zp = pmm.tile([128, PB], FP32, name="zp", tag="mm")
                        for tb in range(T // 128):
                            nc.tensor.matmul(zp[:], tk[tb][:, m * 128:(m + 1) * 128],
                                             maskT[tb][:], start=(tb == 0),
                                             stop=(tb == T // 128 - 1))
                        nc.vector.tensor_copy(spanT[sp][m][:, b * PB:(b + 1) * PB],
                                              zp[:])

        # ---- phase 2: bidirectional chain-LSTM per pair-chunk ----------
        with tc.tile_pool(name="lstmw", bufs=1) as lstmw, \
             tc.tile_pool(name="nodep", bufs=3) as nodep, \
             tc.tile_pool(name="statep", bufs=2) as statep, \
             tc.tile_pool(name="gatep", bufs=16) as gatep, \
             tc.tile_pool(name="eqp", bufs=4) as eqp:
            wu_t = [loadc(lstmw, f"wu{k}", Wu[ko:ko + ksz, :], [ksz, H4], BF16)
                    for k, (ko, ksz) in enumerate(KD_TILES)]
            wd_t = [loadc(lstmw, f"wd{k}", Wd[ko:ko + ksz, :], [ksz, H4], BF16)
                    for k, (ko, ksz) in enumerate(KD_TILES)]
            uu_t = [loadc(lstmw, f"uu{k}", Uu[k * 128:(k + 1) * 128, :], [128, H4],
                          BF16) for k in range(KH)]
            ud_t = [loadc(lstmw, f"ud{k}", Ud[k * 128:(k + 1) * 128, :], [128, H4],
                          BF16) for k in range(KH)]
            w_t = {"u": wu_t, "d": wd_t}
            u_t = {"u": uu_t, "d": ud_t}

            for ch in range(NCH):
                c0 = ch * C
                h16 = {}
                cst = {}
                for d in ("u", "d"):
                    h16[d] = [statep.tile([128, C], BF16, name=f"h_{d}{k}",
                                          tag=f"h_{d}{k}") for k in range(KH)]
                    cst[d] = [statep.tile([128, C], BF16, name=f"c_{d}{k}",
                                          tag=f"c_{d}{k}") for k in range(KH)]
                    for k in range(KH):
                        nc.vector.memset(h16[d][k][:], 0.0)
                        nc.vector.memset(cst[d][k][:], 0.0)
                for k in range(KH):
                    nc.vector.memset(root_acc[ch][k][:], 0.0)

                for s in range(L):
                    for d in ("u", "d"):
                        t_src = s if d == "u" else L - 1 - s
                        nd = []
                        for k, (ko, ksz) in enumerate(KD_TILES):
                            t = nodep.tile([ksz, C], BF16, name=f"nd{k}",
                                           tag=f"nd{k}")
                            nc.sync.dma_start(
                                t[:], node_T[t_src, ko:ko + ksz, c0:c0 + C])
                            nd.append(t)
                        gates = []
                        n_xk = 1 if probe == "xk1" else KD
                        skip_u = probe == "nou"
                        for m in range(M12):
                            pm = pmm.tile([128, C], FP32, name="pm", tag="mm")
                            nk = n_xk if (s == 0 or skip_u) else n_xk + KH
                            for k in range(n_xk):
                                nc.tensor.matmul(
                                    pm[:], w_t[d][k][:, m * 128:(m + 1) * 128],
                                    nd[k][:], start=(k == 0), stop=(k == nk - 1))
                            if s > 0 and not skip_u:
                                for k in range(KH):
                                    nc.tensor.matmul(
                                        pm[:], u_t[d][k][:, m * 128:(m + 1) * 128],
                                        h16[d][k][:], start=False,
                                        stop=(k == KH - 1))
                            if debug and ch == 0 and s == 0 and d == "u" and m == 0:
                                pmc = gatep.tile([128, C], FP32, name="pmc",
                                                 tag="pmc", bufs=1)
                                nc.vector.tensor_copy(pmc[:], pm[:])
                                nc.sync.dma_start(dbg_pm[:, :], pmc[:])
                                for kk, (ko, ksz) in enumerate(KD_TILES):
                                    nc.sync.dma_start(dbg_nd[ko:ko + ksz, :],
                                                      nd[kk][:])
                            g = gatep.tile([128, C], BF16, name="g", tag="g")
                            func = (ACTF.Tanh if 2 * KH <= m < 3 * KH
                                    else ACTF.Sigmoid)
                            nc.scalar.activation(g[:], pm[:], func,
                                                 bias=b_t[d][m][:])
                            if debug and ch == 0 and s == 0 and d == "u":
                                nc.sync.dma_start(dbg_g0[m], g[:])
                            gates.append(g)
                        i_g, o_g, u_g, f_g = (gates[0:3], gates[3:6], gates[6:9],
                                              gates[9:12])
                        hnew = []
                        cnew = []
                        for k in range(KH):
                            tmp = gatep.tile([128, C], BF16, name="tmp", tag="g")
                            nc.vector.tensor_tensor(tmp[:], i_g[k][:], u_g[k][:],
                                                    ALU.mult)
                            cn = statep.tile([128, C], BF16, name=f"cn_{d}{k}",
                                             tag=f"c_{d}{k}")
                            if s == 0:
                                nc.vector.tensor_copy(cn[:], tmp[:])
                            else:
                                nc.vector.tensor_tensor(cn[:], f_g[k][:],
                                                        cst[d][k][:], ALU.mult)
                                nc.vector.tensor_tensor(cn[:], cn[:], tmp[:],
                                                        ALU.add)
                            tc_ = gatep.tile([128, C], BF16, name="tc", tag="g")
                            nc.scalar.activation(tc_[:], cn[:], ACTF.Tanh)
                            hn = statep.tile([128, C], BF16, name=f"hn_{d}{k}",
                                             tag=f"h_{d}{k}")
                            nc.vector.tensor_tensor(hn[:], o_g[k][:], tc_[:],
                                                    ALU.mult)
                            hnew.append(hn)
                            cnew.append(cn)
                        h16[d] = hnew
                        cst[d] = cnew
                        if debug and ch == 0 and s == 0 and d == "u":
                            for k in range(KH):
                                nc.sync.dma_start(dbg_h1[k], hnew[k][:])
                        if d == "u":
                            eq = eqp.tile([1, C], BF16, name="eq", tag="eq")
                            nc.vector.tensor_scalar(eq[:], root_t[:, c0:c0 + C],
                                                    float(s), None, ALU.is_equal)
                            mp = pmask.tile([128, C], FP32, name="mp", tag="mp")
                            nc.tensor.matmul(mp[:], ones_t[:], eq[:], start=True,
                                             stop=True)
                            # walrus requires an integer mask for CopyPredicated
                            mpi = eqp.tile([128, C], mybir.dt.uint8, name="mpi",
                                           tag="mpi", bufs=2)
                            nc.vector.tensor_copy(mpi[:], mp[:])
                            for k in range(KH):
                                nc.vector.copy_predicated(root_acc[ch][k][:],
                                                          mpi[:],
                                                          h16["u"][k][:])
                        else:
                            if s == 0:
                                for k in range(KH):
                                    end_t[ch][k] = capp.tile(
                                        [128, C], BF16, name=f"end{ch}_{k}",
                                        tag=f"end{ch}_{k}")
                                    nc.vector.tensor_copy(end_t[ch][k][:],
                                                          h16["d"][k][:])
                            if s == L - 1:
                                for k in range(KH):
                                    start_t[ch][k] = capp.tile(
                                        [128, C], BF16, name=f"start{ch}_{k}",
                                        tag=f"start{ch}_{k}")
                                    nc.vector.tensor_copy(start_t[ch][k][:],
                                                          h16["d"][k][:])

        if debug:
            for sp in range(2):
                for m in range(MT):
                    nc.sync.dma_start(dbg_span[sp, m], spanT[sp][m][:])
            for ch in range(NCH):
                for k in range(KH):
                    nc.sync.dma_start(dbg_racc[ch, k], root_acc[ch][k][:])
                    nc.sync.dma_start(dbg_start[ch, k], start_t[ch][k][:])
                    nc.sync.dma_start(dbg_end[ch, k], end_t[ch][k][:])

        # ---- phase 3: pair MLP -----------------------------------------
        with tc.tile_pool(name="mlpw", bufs=1) as mlpw, \
             tc.tile_pool(name="mlpp", bufs=4) as mlpp:
            w1_t = [loadc(mlpw, f"w1{k}", W1[k * 128:(k + 1) * 128, :],
                          [128, DEC_H], BF16) for k in range(K21)]
            w2_t = [loadc(mlpw, f"w2{k}", W2[k * 128:(k + 1) * 128, :],
                          [128, DEC_OUT], BF16) for k in range(M4)]
            for ch in range(NCH):
                c0 = ch * C
                feats = (root_acc[ch] + start_t[ch] + end_t[ch]
                         + [spanT[0][m][:, c0:c0 + C] for m in range(MT)]
                         + [spanT[1][m][:, c0:c0 + C] for m in range(MT)])
                z_t = []
                for m in range(M4):
                    zp = pmm.tile([128, C], FP32, name="zp2", tag="mm")
                    for k in range(K21):
                        fk = feats[k] if isinstance(feats[k], bass.AP) \
                            else feats[k][:]
                        nc.tensor.matmul(zp[:], w1_t[k][:, m * 128:(m + 1) * 128],
                                         fk, start=(k == 0), stop=(k == K21 - 1))
                    z = mlpp.tile([128, C], BF16, name="z", tag="z")
                    nc.scalar.activation(z[:], zp[:], ACTF.Tanh, bias=b1_t[m][:])
                    z_t.append(z)
                op = pout.tile([DEC_OUT, C], FP32, name="op", tag="op")
                for m in range(M4):
                    nc.tensor.matmul(op[:], w2_t[m][:], z_t[m][:], start=(m == 0),
                                     stop=(m == M4 - 1))
                osb = mlpp.tile([DEC_OUT, C], FP32, name="osb", tag="osb", bufs=2)
                nc.vector.tensor_scalar(osb[:], op[:], b2_t[:], None, ALU.add)
                nc.sync.dma_start(out_d[:, c0:c0 + C], osb[:])

    nc.compile()
    return nc


_CACHE = {}


def _get_program() -> bass.Bass:
    if "nc" not in _CACHE:
        _CACHE["nc"] = _build_program()
    return _CACHE["nc"]


def _prep_in_maps(inputs) -> list[dict]:
    f32 = np.float32
    node = np.asarray(inputs["node_embs"], f32)
    tokf = np.asarray(inputs["token_embs"], f32)
    rooti = np.asarray(inputs["root_idx"])
    # [P, L, D] -> per-core [L, D, PS] bf16
    node_sh = np.ascontiguousarray(
        node.reshape(NCORES, PS, L, D).transpose(0, 2, 3, 1)).astype(bf16)
    tok_sh = tokf.reshape(NCORES, NB, T, DT).astype(bf16)
    root_sh = rooti.reshape(NCORES, 1, PS).astype(f32)

    def span_arrays(st, ln):
        st = np.asarray(st).astype(f32)
        ln = np.asarray(ln).astype(f32)
        en = st + ln + 1.0
        rc = 1.0 / (ln + 1.0)
        return st, en, rc

    s1, e1, r1 = span_arrays(inputs["p1_st"], inputs["p1_len"])
    s2, e2, r2 = span_arrays(inputs["p2_st"], inputs["p2_len"])

    def pack_span(a1, a2):
        # [B, PB] x2 -> per-core [2, NB, JT, 128]
        a = np.stack([a1, a2])  # [2, B, PB]
        a = a.reshape(2, NCORES, NB, JT, 128).transpose(1, 0, 2, 3, 4)
        return np.ascontiguousarray(a.astype(f32))

    stp, enp, rcp = pack_span(s1, s2), pack_span(e1, e2), pack_span(r1, r2)
    zp = np.zeros_like(stp)
    # [NCORES, 2, NB, JT, 128, 4]: st | en | recip | pad
    sp_all = np.ascontiguousarray(np.stack([stp, enp, rcp, zp], axis=-1))

    Wu_h = np.concatenate([np.asarray(inputs["Wiou_u"], f32),
                           np.asarray(inputs["Wf_u"], f32)], axis=1).astype(bf16)
    Wd_h = np.concatenate([np.asarray(inputs["Wiou_d"], f32),
                           np.asarray(inputs["Wf_d"], f32)], axis=1).astype(bf16)
    Uu_h = np.concatenate([np.asarray(inputs["Uiou_u"], f32),
                           np.asarray(inputs["Uf_u"], f32)], axis=1).astype(bf16)
    Ud_h = np.concatenate([np.asarray(inputs["Uiou_d"], f32),
                           np.asarray(inputs["Uf_d"], f32)], axis=1).astype(bf16)
    bu_h = np.concatenate([np.asarray(inputs["biou_u"], f32),
                           np.asarray(inputs["bf_u"], f32)]).reshape(M12, 128, 1)
    bd_h = np.concatenate([np.asarray(inputs["biou_d"], f32),
                           np.asarray(inputs["bf_d"], f32)]).reshape(M12, 128, 1)
    W1_h = np.asarray(inputs["W1"], f32).astype(bf16)
    W2_h = np.asarray(inputs["W2"], f32).astype(bf16)
    b1_h = np.asarray(inputs["b1"], f32).reshape(M4, 128, 1)
    b2_h = np.asarray(inputs["b2"], f32).reshape(DEC_OUT, 1)
    ones_h = np.ones((1, 128), bf16)
    iota_h = np.broadcast_to(np.arange(T, dtype=f32), (128, T)).copy()

    in_maps = []
    for c in range(NCORES):
        in_maps.append({
            "node_T": node_sh[c], "tok": tok_sh[c], "root": root_sh[c],
            "sp_all": sp_all[c],
            "Wu": Wu_h, "Wd": Wd_h, "Uu": Uu_h, "Ud": Ud_h,
            "W1": W1_h, "W2": W2_h, "bu": bu_h, "bd": bd_h,
            "b1": b1_h, "b2": b2_h, "ones": ones_h, "iota_d": iota_h,
        })
    return in_maps


def run(inputs, **kwargs):
    """Run on hardware; returns (output [P, DEC_OUT] fp32, BassKernelResults)."""
    nc = _get_program()
    in_maps = _prep_in_maps(inputs)
    res = run_bass_kernel_spmd(nc, in_maps, list(range(NCORES)), **kwargs)
    outs = [np.asarray(r["out"], np.float32).T for r in res.results]  # [PS, 7] each
    return np.concatenate(outs, axis=0), res


def kernel(**inputs) -> np.ndarray:
    out, _ = run(inputs)
    return out
